# revision 24
# baseline (speedup 1.0000x reference)
"""CRF forward kernel, v4: fp8e5 DoubleRow, zero-halo chunks.

Forward recurrence in rescaled linear space with a constant per-step
shift c = log(N)+0.505 folded into exp(u) (restored analytically as
T*c): p' = diag(exp(u-c)) E p. The T=65536 chain is cut into 4096
chunks of L=16 steps, each started directly from a ones vector (W=0):
the transition matrix contracts directions ~200x per step, so the
start-direction error contributes ~3e-4 per chunk to logZ (validated
by emulation); the chunk-start normalizer is then the KNOWN constant
S0 = N (and 1 for chunk 0, which starts from its exact one-hot shipped
in the initial-state input). Each chunk's contribution is
log(S_end/S0) + L*c with S_end measured by a ones/tau two-row matmul
at the last step only.

Per core per step: 32 DoubleRow matmuls (fp8e5, stationary E^T pairs,
moving state q[j, b], B=512 columns; 2 fp8/cycle stream, LDWEIGHTS
hidden), 8 DVE multiplies by the host-precomputed exp(u-c) (bf16).
Per-group psum tiles and per-pair q tiles keep Tile's dependency
tracking fine-grained: the DVE multiplies interleave with the matmul
stream and the PE never idles (HAM stays at 8/8). E^T is loaded in 8
column-chunks so the first matmul group only waits for 128KB.
"""

import math

import numpy as np
import ml_dtypes
from contextlib import ExitStack

T = 65536
N = 1024
NCORES = 8
B = 512           # chunk-columns per core (matmul moving dim)
L = 16            # chunk length (steps whose growth this chunk owns)
W = 0             # no halo: chunks start from ones (S0 known)
STEPS = L
PERCORE = T // NCORES
C_SHIFT = math.log(N) + 0.505   # per-step rescale, restored as +T*C_SHIFT
BOOST = math.log(512.0)         # chunk-0 first-row boost, subtracted on host
HOST_EXP = True                 # "u" input already holds exp(u - c)

_BF = ml_dtypes.bfloat16
_F8 = ml_dtypes.float8_e5m2

_compiled = {}


def _build_bass():
    import concourse.bacc as bacc
    import concourse.tile as tile
    from concourse import mybir

    bf = mybir.dt.bfloat16
    f8 = mybir.dt.float8e5
    f32 = mybir.dt.float32
    DR = mybir.MatmulPerfMode.DoubleRow

    nc = bacc.Bacc("TRN2", name="crf_fwd4")

    U = nc.dram_tensor("u", [STEPS, 128, 8, B], bf, kind="ExternalInput")
    ET = nc.dram_tensor("et", [8, 128, 8, 128], f8, kind="ExternalInput")
    TAU2 = nc.dram_tensor("tau2", [128, 8, 2], f8, kind="ExternalInput")
    RS = nc.dram_tensor("rs", [128, 8], f32, kind="ExternalInput")
    OUT_SE = nc.dram_tensor("se", [2, B], f32, kind="ExternalOutput")

    with tile.TileContext(nc) as tc, ExitStack() as ctx:
        consts = ctx.enter_context(tc.tile_pool(name="consts", bufs=1))
        upool = ctx.enter_context(tc.tile_pool(name="u", bufs=3))
        qpool = ctx.enter_context(tc.tile_pool(name="q", bufs=2))
        srows = ctx.enter_context(tc.tile_pool(name="srows", bufs=1))
        ps_mm = ctx.enter_context(tc.tile_pool(name="psmm", bufs=1, space="PSUM"))

        # et_sb[p, it, jt, i2] = E^T[jt*128+p, it*128+i2]; the DoubleRow
        # stationary AP for (it, jd) is et_sb[:, it, 2jd:2jd+2, :].
        # Loaded in 8 it-chunks (contiguous 1KB/partition each) so the
        # first matmul group waits only for chunk 0.
        # u[0] first on gpsimd: step 0 is computed on DVE only (from a
        # ones init, q1 = eu0 * rowsum(E) -- no matmuls), so the PE's
        # first need is et chunks for step 1, giving the load ~9us slack.
        eut0 = upool.tile([128, 8, B], bf, tag="eut", name="eut0")
        nc.gpsimd.dma_start(out=eut0[:], in_=U[0])

        rs_sb = consts.tile([128, 8], f32)
        nc.sync.dma_start(out=rs_sb[:], in_=RS.ap())

        et_sb = consts.tile([128, 8, 8, 128], f8)
        for it in range(8):
            eng = nc.sync if it % 2 == 0 else nc.gpsimd
            eng.dma_start(out=et_sb[:, it, :, :], in_=ET.ap()[it])

        # sm[p, jt, m]: m=0 -> ones row, m=1 -> tau row (exp trans[end]);
        # loaded late (only needed by the final S-measure)
        sm = consts.tile([128, 8, 2], f8)

        # initial q: ones for every chunk; chunk 0's one-hot start is
        # folded into its first eu row on the host (E[:,start]/rowsum)
        q_init = []
        for jd in range(4):
            qi = consts.tile([128, 2, B], f8, tag=f"qi{jd}", name=f"qi{jd}")
            nc.vector.memset(qi[:], 1.0)
            q_init.append(qi)

        se_row = srows.tile([2, B], f32, tag="serow")

        qcur = [t[:] for t in q_init]
        for s in range(STEPS):
            if s == 0:
                eut = eut0
            else:
                eut = upool.tile([128, 8, B], bf, tag="eut")
                dma_eng = nc.gpsimd if s % 2 == 0 else nc.sync
                dma_eng.dma_start(out=eut[:], in_=U[s])

            qnext = [qpool.tile([128, 2, B], f8, tag=f"qn{i}", name=f"qn{i}") for i in range(4)]
            if s == 0:
                # ones init: E @ 1 = rowsum, so step 0 needs no matmuls
                for it in range(8):
                    nc.vector.tensor_scalar_mul(
                        qnext[it // 2][:, it % 2, :],
                        eut[:, it, :],
                        rs_sb[:, it : it + 1],
                    )
            else:
                psums = [ps_mm.tile([128, B], f32, tag=f"ps{i}", name=f"ps{i}") for i in range(8)]
                for it in range(8):
                    for jd in range(4):
                        nc.tensor.matmul(
                            psums[it][:],
                            et_sb[:, it, 2 * jd : 2 * jd + 2, :],
                            qcur[jd][:],
                            start=(jd == 0),
                            stop=(jd == 3),
                            perf_mode=DR,
                        )
                    nc.vector.tensor_mul(
                        qnext[it // 2][:, it % 2, :], psums[it][:], eut[:, it, :]
                    )
            qcur = [t[:] for t in qnext]

            if s == STEPS - 1:
                nc.sync.dma_start(out=sm[:], in_=TAU2.ap())
                # reuse the ps0 slot (bank 0) for the chunk-normalizer row
                ps = ps_mm.tile([2, B], f32, tag="ps0", name="pssum")
                for jt in range(8):
                    nc.tensor.matmul(
                        ps[:],
                        sm[:, jt, :],
                        qnext[jt // 2][:, jt % 2, :],
                        start=(jt == 0),
                        stop=(jt == 7),
                    )
                nc.vector.tensor_copy(out=se_row[:], in_=ps[:])
                nc.sync.dma_start(out=OUT_SE.ap(), in_=se_row[:])

    nc.finalize()
    return nc


def _get_nc():
    if "nc" not in _compiled:
        _compiled["nc"] = _build_bass()
    return _compiled["nc"]


def _prep_inputs(unary, transitions, start_idx, end_idx):
    """Host-side: exp + casts + per-core gather into [STEPS, 128, 8, B]."""
    unary = np.asarray(unary, dtype=np.float32)
    transitions = np.asarray(transitions, dtype=np.float32)

    # et[it, p, jt, i2] = E^T[jt*128+p, it*128+i2]
    etm = np.exp(transitions).T  # [j, i]
    et = etm.reshape(8, 128, 8, 128).transpose(2, 1, 0, 3)
    et = np.ascontiguousarray(et).astype(_F8)

    tau2 = np.empty((128, 8, 2), dtype=np.float32)
    tau2[:, :, 0] = 1.0
    tau2[:, :, 1] = np.exp(transitions[end_idx]).reshape(8, 128).T
    tau2 = tau2.astype(_F8)

    rs = N * 4  # f32 row stride in bytes
    in_maps = []
    for c in range(NCORES):
        base = unary[c * PERCORE :]
        view = np.lib.stride_tricks.as_strided(
            base, shape=(B, STEPS, N), strides=(L * rs, rs, 4)
        )
        # [B, STEPS, N] -> [STEPS, 128(p), 8(it), B];  i = it*128 + p
        ucore = view.transpose(1, 2, 0).reshape(STEPS, 8, 128, B)
        ucore = np.ascontiguousarray(ucore.transpose(0, 2, 1, 3))
        if c == 0:
            # chunk 0's first unary row is boosted so the spread state
            # lands at O(1) mean in fp8 (subtracted in _combine)
            ucore[0, :, :, 0] += BOOST
        eucore = np.exp(ucore - C_SHIFT)
        if c == 0:
            # fold chunk 0's exact one-hot start into its first eu row:
            # from a ones init, eu*E[:,start]/rowsum reproduces the true
            # (unnormalized, S0 = 1) first state exactly
            E = np.exp(transitions)
            factor = E[:, start_idx] / E.sum(axis=1)  # [i]
            eucore[0, :, :, 0] *= factor.reshape(8, 128).T
        rs_arr = np.ascontiguousarray(
            np.exp(transitions).sum(axis=1).reshape(8, 128).T
        ).astype(np.float32)
        in_maps.append({"u": eucore.astype(_BF), "et": et, "tau2": tau2, "rs": rs_arr})
    return in_maps


def _combine(results):
    # sum over chunks of log(S_end/S0) + T*c - boost; S0 = N for every
    # chunk except chunk 0 (exact one-hot, S0 = 1)
    nchunks = NCORES * B
    tot = float(T) * C_SHIFT - BOOST - (nchunks - 1) * math.log(float(N))
    for r in results:
        se = r["se"].astype(np.float64)
        tot += float(np.sum(np.log(se[0])))
    last = results[-1]["se"].astype(np.float64)
    tot += float(np.log(last[1, B - 1]) - np.log(last[0, B - 1]))
    return tot


def kernel(unary, transitions, start_idx, end_idx, _trace=False):
    from concourse.bass_utils import run_bass_kernel_spmd

    start_idx = int(np.asarray(start_idx))
    end_idx = int(np.asarray(end_idx))

    nc = _get_nc()
    in_maps = _prep_inputs(unary, transitions, start_idx, end_idx)
    res = run_bass_kernel_spmd(nc, in_maps, core_ids=list(range(NCORES)), trace=_trace)
    _compiled["last_result"] = res
    logZ = _combine(res.results)
    return np.array(logZ, dtype=np.float32)
